# revision 15
# baseline (speedup 1.0000x reference)
"""PerNeuronMLPHead Trainium2 kernel.

out[b,t,n] = clip(w3 . gelu(W2^T gelu(a[b,t,:] + u[n,:] + b1) + b2) + b3, -10, 10)
  a = (bin_repr @ bp_w + bp_b) @ w1[:H]     # per-token part,  [B*T, H]
  u = (unit_embs @ up_w + up_b) @ w1[H:]    # per-neuron part, [N, H]

Sharding: neuron axis N=256 split over 8 cores (32 each); all weights and
bin_repr replicated. Everything on-chip is feature-on-partition so the
broadcast add a+u fuses into the ACT gelu bias operand.
"""

import numpy as np
from contextlib import ExitStack

import ml_dtypes
import concourse.bass as bass
import concourse.tile as tile
from concourse import bacc, mybir
from concourse.bass_utils import run_bass_kernel_spmd

F32 = mybir.dt.float32
F32R = mybir.dt.float32r
BF16 = mybir.dt.bfloat16
GELU = mybir.ActivationFunctionType.Gelu
ALU = mybir.AluOpType

# Problem constants (hardcoded per contest rules)
B, T, DIM = 2, 512, 512
N_NEURONS = 256
HALF = DIM // 2          # 256
QUART = HALF // 2        # 128
R = B * T                # 1024 tokens
N_CORES = 8
NPC = N_NEURONS // N_CORES  # 32 neurons per core
RC = 512                 # r-chunk (psum bank limit for fp32 moving dim)
P = 128

MM_DT = F32R             # dtype for the main matmul operands
H2_DT = F32              # dtype for h2 (stationary of the w3 matvec) + w3

_CACHE = {}


def _build_body(nc, tc, pools, d, out_d, rep):
    wsb, act, h2p, psA, ps2, psO = pools
    rp = f"r{rep}_"

    # ---- load inputs to SBUF ----
    def load_tiles(ap_d, k_tiles, cols, dt, tag):
        ts = []
        for k in range(k_tiles):
            t = wsb.tile([P, cols], dt, tag=f"{tag}{k}", name=f"{rp}{tag}{k}")
            nc.sync.dma_start(t[:], ap_d[k * P:(k + 1) * P, :])
            ts.append(t)
        return ts

    # binT loaded in r-half chunks, critical-path pieces first: the first
    # bin_hT psum rounds need binT[:, 0:512] + bp_w + w1a only.
    binT = [wsb.tile([P, R], MM_DT, tag=f"binT{k}", name=f"{rp}binT{k}")
            for k in range(4)]
    for k in range(4):
        nc.sync.dma_start(binT[k][:, 0:RC], d["binT"][k * P:(k + 1) * P, 0:RC])
    bp_w = load_tiles(d["bp_w"], 4, HALF, MM_DT, "bp_w")
    w1a = load_tiles(d["w1a"], 2, HALF, MM_DT, "w1a")
    for k in range(4):
        nc.sync.dma_start(binT[k][:, RC:R], d["binT"][k * P:(k + 1) * P, RC:R])
    unitT = load_tiles(d["unitT"], 4, NPC, MM_DT, "unitT")
    up_w = load_tiles(d["up_w"], 4, HALF, MM_DT, "up_w")
    w1b = load_tiles(d["w1b"], 2, HALF, MM_DT, "w1b")
    w2 = load_tiles(d["w2"], 2, QUART, MM_DT, "w2")
    w3 = wsb.tile([QUART, 1], H2_DT, tag="w3", name=f"{rp}w3")
    nc.sync.dma_start(w3[:], d["w3"][:])

    def load_bias(name, k_tiles):
        ts = []
        for k in range(k_tiles):
            t = wsb.tile([P, 1], F32, tag=f"{name}{k}", name=f"{rp}{name}{k}")
            nc.sync.dma_start(t[:], d[name][k] if k_tiles > 1 else d[name][:])
            ts.append(t)
        return ts

    bp_b = load_bias("bp_b", 2)
    up_b = load_bias("up_b", 2)
    b1v = load_bias("b1v", 2)
    b2v = load_bias("b2v", 1)[0]
    b3v = load_bias("b3v", 1)[0]

    # ---- stage A: per-token and per-neuron linear parts ----
    # bin_hT[h, r] = sum_d bp_w[d, h] * binT[d, r]  (+ bp_b)
    bin_hT = [wsb.tile([P, R], MM_DT, tag=f"bin_hT{m}", name=f"{rp}bin_hT{m}")
              for m in range(2)]
    aT = [wsb.tile([P, R], F32, tag=f"aT{m}", name=f"{rp}aT{m}")
          for m in range(2)]
    for rc in range(R // RC):
        for m in range(2):
            p = psA.tile([P, RC], F32, tag="psA", name=f"{rp}pA")
            for k in range(4):
                nc.tensor.matmul(
                    p[:], bp_w[k][:, m * P:(m + 1) * P],
                    binT[k][:, rc * RC:(rc + 1) * RC],
                    start=(k == 0), stop=(k == 3))
            nc.vector.tensor_scalar_add(
                bin_hT[m][:, rc * RC:(rc + 1) * RC], p[:], bp_b[m][:])
        # aT[f, r] = sum_h w1a[h, f] * bin_hT[h, r]
        for m in range(2):
            p = psA.tile([P, RC], F32, tag="psA", name=f"{rp}pA")
            for k in range(2):
                nc.tensor.matmul(
                    p[:], w1a[k][:, m * P:(m + 1) * P],
                    bin_hT[k][:, rc * RC:(rc + 1) * RC],
                    start=(k == 0), stop=(k == 1))
            nc.vector.tensor_copy(aT[m][:, rc * RC:(rc + 1) * RC], p[:])

    # unit_hT[h, n] = sum_d up_w[d, h] * unitT[d, n]  (+ up_b)
    unit_hT = [wsb.tile([P, NPC], MM_DT, tag=f"unit_hT{m}", name=f"{rp}unit_hT{m}")
               for m in range(2)]
    for m in range(2):
        p = psA.tile([P, RC], F32, tag="psA", name=f"{rp}pA")
        for k in range(4):
            nc.tensor.matmul(
                p[:, :NPC], up_w[k][:, m * P:(m + 1) * P], unitT[k][:],
                start=(k == 0), stop=(k == 3))
        nc.vector.tensor_scalar_add(unit_hT[m][:], p[:, :NPC], up_b[m][:])

    # uT[f, n] = sum_h w1b[h, f] * unit_hT[h, n]  (+ b1)
    uT = [wsb.tile([P, NPC], F32, tag=f"uT{m}", name=f"{rp}uT{m}")
          for m in range(2)]
    for m in range(2):
        p = psA.tile([P, RC], F32, tag="psA", name=f"{rp}pA")
        for k in range(2):
            nc.tensor.matmul(
                p[:, :NPC], w1b[k][:, m * P:(m + 1) * P], unit_hT[k][:],
                start=(k == 0), stop=(k == 1))
        nc.vector.tensor_scalar_add(uT[m][:], p[:, :NPC], b1v[m][:])

    # ---- stage B: per-neuron MLP ----
    ps_out = psO.tile([P, 8 * NPC], F32, tag="ps_out", name=f"{rp}ps_out")

    GN = 4  # neurons per gelu1 group
    for grp in range(NPC // GN):
        # z[f, j*R + r] = aT[f, r] + uT[f, grp*GN+j]  (DVE), then one big gelu
        h1 = []
        for m in range(2):
            # split the broadcast-adds between DVE (m=0) and GPSIMD (m=1)
            add_eng = nc.vector if m == 0 else nc.gpsimd
            z = act.tile([P, GN * R], F32, tag=f"zh{m}", name=f"{rp}z{m}_{grp}")
            for j in range(GN):
                add_eng.tensor_scalar_add(
                    z[:, j * R:(j + 1) * R], aT[m][:],
                    uT[m][:, grp * GN + j: grp * GN + j + 1])
            h = act.tile([P, GN * R], MM_DT, tag=f"zh{m}", name=f"{rp}h1_{m}_{grp}")
            nc.scalar.activation(h[:], z[:], GELU)
            h1.append(h)

        for j in range(GN):
            n = grp * GN + j
            # h2pre[g, r] = sum_f w2[f, g] h1[f, r]
            p2 = ps2.tile([P, R], F32, tag="p2", name=f"{rp}p2_{n}")
            for rc in range(R // RC):
                for k in range(2):
                    nc.tensor.matmul(
                        p2[:, rc * RC:(rc + 1) * RC], w2[k][:],
                        h1[k][:, j * R + rc * RC: j * R + (rc + 1) * RC],
                        start=(k == 0), stop=(k == 1))
            # h2 = gelu(h2pre + b2)
            h2 = h2p.tile([P, R], H2_DT, tag="h2", name=f"{rp}h2_{n}")
            nc.scalar.activation(h2[:], p2[:], GELU, bias=b2v[:])
            # out[:, rt*32+n] = h2[:, rt*128:...]^T @ w3
            for rt in range(8):
                nc.tensor.matmul(
                    ps_out[:, rt * NPC + n: rt * NPC + n + 1],
                    h2[:, rt * P:(rt + 1) * P], w3[:],
                    start=True, stop=True)

    # ---- epilogue: +b3, clip, store ----
    ob = wsb.tile([P, 8 * NPC], F32, tag="ob", name=f"{rp}ob")
    nc.vector.tensor_scalar(ob[:], ps_out[:], b3v[:], -10.0,
                            op0=ALU.add, op1=ALU.max)
    nc.vector.tensor_scalar_min(ob[:], ob[:], 10.0)
    nc.sync.dma_start(
        out_d.rearrange("(t p) n -> p t n", p=P),
        ob[:].rearrange("p (t n) -> p t n", t=8))


def build_program(reps=1):
    nc = bacc.Bacc("TRN2", target_bir_lowering=False, debug=False,
                   num_devices=N_CORES)

    d = {}
    d["binT"] = nc.dram_tensor("binT", [DIM, R], MM_DT, kind="ExternalInput").ap()
    d["unitT"] = nc.dram_tensor("unitT", [DIM, NPC], MM_DT, kind="ExternalInput").ap()
    d["bp_w"] = nc.dram_tensor("bp_w", [DIM, HALF], MM_DT, kind="ExternalInput").ap()
    d["up_w"] = nc.dram_tensor("up_w", [DIM, HALF], MM_DT, kind="ExternalInput").ap()
    d["w1a"] = nc.dram_tensor("w1a", [HALF, HALF], MM_DT, kind="ExternalInput").ap()
    d["w1b"] = nc.dram_tensor("w1b", [HALF, HALF], MM_DT, kind="ExternalInput").ap()
    d["w2"] = nc.dram_tensor("w2", [HALF, QUART], MM_DT, kind="ExternalInput").ap()
    d["w3"] = nc.dram_tensor("w3", [QUART, 1], H2_DT, kind="ExternalInput").ap()
    d["bp_b"] = nc.dram_tensor("bp_b", [2, P, 1], F32, kind="ExternalInput").ap()
    d["up_b"] = nc.dram_tensor("up_b", [2, P, 1], F32, kind="ExternalInput").ap()
    d["b1v"] = nc.dram_tensor("b1v", [2, P, 1], F32, kind="ExternalInput").ap()
    d["b2v"] = nc.dram_tensor("b2v", [P, 1], F32, kind="ExternalInput").ap()
    d["b3v"] = nc.dram_tensor("b3v", [P, 1], F32, kind="ExternalInput").ap()
    out_d = nc.dram_tensor("out", [R, NPC], F32, kind="ExternalOutput").ap()

    with tile.TileContext(nc) as tc:
        with ExitStack() as ctx:
            wsb = ctx.enter_context(tc.tile_pool(name="wsb", bufs=1))
            act = ctx.enter_context(tc.tile_pool(name="act", bufs=3))
            h2p = ctx.enter_context(tc.tile_pool(name="h2p", bufs=2))
            psA = ctx.enter_context(tc.tile_pool(name="psA", bufs=2, space="PSUM"))
            ps2 = ctx.enter_context(tc.tile_pool(name="ps2", bufs=2, space="PSUM"))
            psO = ctx.enter_context(tc.tile_pool(name="psO", bufs=1, space="PSUM"))
            pools = (wsb, act, h2p, psA, ps2, psO)
            for rep in range(reps):
                _build_body(nc, tc, pools, d, out_d, rep)

    nc.compile()
    return nc


def _make_in_maps(bin_repr, unit_embs, bp_w, bp_b, up_w, up_b, w1, b1, w2, b2,
                  w3, b3):
    f32 = np.float32
    w3_np = np.ascontiguousarray(w3, f32)
    if H2_DT == BF16:
        w3_np = w3_np.astype(ml_dtypes.bfloat16)
    binT = np.ascontiguousarray(bin_repr.reshape(R, DIM).T).astype(f32)
    common = {
        "binT": binT,
        "bp_w": np.ascontiguousarray(bp_w, f32),
        "up_w": np.ascontiguousarray(up_w, f32),
        "w1a": np.ascontiguousarray(w1[:HALF], f32),
        "w1b": np.ascontiguousarray(w1[HALF:], f32),
        "w2": np.ascontiguousarray(w2, f32),
        "w3": w3_np,
        "bp_b": np.ascontiguousarray(bp_b, f32).reshape(2, P, 1),
        "up_b": np.ascontiguousarray(up_b, f32).reshape(2, P, 1),
        "b1v": np.ascontiguousarray(b1, f32).reshape(2, P, 1),
        "b2v": np.ascontiguousarray(b2, f32).reshape(P, 1),
        "b3v": np.full((P, 1), np.float32(np.asarray(b3).reshape(-1)[0]), f32),
    }
    in_maps = []
    for c in range(N_CORES):
        m = dict(common)
        m["unitT"] = np.ascontiguousarray(
            unit_embs[c * NPC:(c + 1) * NPC].T).astype(f32)
        in_maps.append(m)
    return in_maps


def _gather(res):
    parts = [res.results[c]["out"] for c in range(N_CORES)]  # each [R, NPC]
    full = np.concatenate(parts, axis=1)                     # [R, N]
    return full.reshape(B, T, N_NEURONS).astype(np.float32)


def kernel(**inputs):
    if "nc" not in _CACHE:
        _CACHE["nc"] = build_program()
    in_maps = _make_in_maps(**{k: np.asarray(v) for k, v in inputs.items()})
    res = run_bass_kernel_spmd(_CACHE["nc"], in_maps,
                               core_ids=list(range(N_CORES)))
    return _gather(res)


# revision 16
# speedup vs baseline: 3.9536x; 3.9536x over previous
"""PerNeuronMLPHead Trainium2 kernel.

out[b,t,n] = clip(w3 . gelu(W2^T gelu(a[b,t,:] + u[n,:] + b1) + b2) + b3, -10, 10)
  a = (bin_repr @ bp_w + bp_b) @ w1[:H]     # per-token part,  [B*T, H]
  u = (unit_embs @ up_w + up_b) @ w1[H:]    # per-neuron part, [N, H]

Sharding: neuron axis N=256 split over 8 cores (32 each); all weights and
bin_repr replicated. Everything on-chip is feature-on-partition so the
broadcast add a+u fuses into the ACT gelu bias operand.
"""

import numpy as np
from contextlib import ExitStack

import ml_dtypes
import concourse.bass as bass
import concourse.tile as tile
from concourse import bacc, mybir
from concourse.bass_utils import run_bass_kernel_spmd

F32 = mybir.dt.float32
F32R = mybir.dt.float32r
BF16 = mybir.dt.bfloat16
GELU = mybir.ActivationFunctionType.Gelu
ALU = mybir.AluOpType

# Problem constants (hardcoded per contest rules)
B, T, DIM = 2, 512, 512
N_NEURONS = 256
HALF = DIM // 2          # 256
QUART = HALF // 2        # 128
R = B * T                # 1024 tokens
N_CORES = 8
NPC = N_NEURONS // N_CORES  # 32 neurons per core
RC = 512                 # r-chunk (psum bank limit for fp32 moving dim)
P = 128

MM_DT = F32R             # dtype for the main matmul operands
H2_DT = F32              # dtype for h2 (stationary of the w3 matvec) + w3

_CACHE = {}


def _build_body(nc, tc, pools, d, out_d, rep):
    wsb, act, h2p, psA, ps2, psO = pools
    rp = f"r{rep}_"

    # ---- load inputs to SBUF ----
    def load_tiles(ap_d, k_tiles, cols, dt, tag):
        ts = []
        for k in range(k_tiles):
            t = wsb.tile([P, cols], dt, tag=f"{tag}{k}", name=f"{rp}{tag}{k}")
            nc.sync.dma_start(t[:], ap_d[k * P:(k + 1) * P, :])
            ts.append(t)
        return ts

    # binT loaded in r-half chunks, critical-path pieces first: the first
    # bin_hT psum rounds need binT[:, 0:512] + bp_w + w1a only.
    binT = [wsb.tile([P, R], MM_DT, tag=f"binT{k}", name=f"{rp}binT{k}")
            for k in range(4)]
    for k in range(4):
        nc.sync.dma_start(binT[k][:, 0:RC], d["binT"][k * P:(k + 1) * P, 0:RC])
    bp_w = load_tiles(d["bp_w"], 4, HALF, MM_DT, "bp_w")
    w1a = load_tiles(d["w1a"], 2, HALF, MM_DT, "w1a")
    for k in range(4):
        nc.sync.dma_start(binT[k][:, RC:R], d["binT"][k * P:(k + 1) * P, RC:R])
    unitT = load_tiles(d["unitT"], 4, NPC, MM_DT, "unitT")
    up_w = load_tiles(d["up_w"], 4, HALF, MM_DT, "up_w")
    w1b = load_tiles(d["w1b"], 2, HALF, MM_DT, "w1b")
    w2 = load_tiles(d["w2"], 2, QUART, MM_DT, "w2")
    w3 = wsb.tile([QUART, 1], H2_DT, tag="w3", name=f"{rp}w3")
    nc.sync.dma_start(w3[:], d["w3"][:])

    def load_bias(name, k_tiles):
        ts = []
        for k in range(k_tiles):
            t = wsb.tile([P, 1], F32, tag=f"{name}{k}", name=f"{rp}{name}{k}")
            nc.sync.dma_start(t[:], d[name][k] if k_tiles > 1 else d[name][:])
            ts.append(t)
        return ts

    bp_b = load_bias("bp_b", 2)
    up_b = load_bias("up_b", 2)
    b1v = load_bias("b1v", 2)
    b2v = load_bias("b2v", 1)[0]
    b3v = load_bias("b3v", 1)[0]

    # ---- stage A: per-token and per-neuron linear parts ----
    # bin_hT[h, r] = sum_d bp_w[d, h] * binT[d, r]  (+ bp_b)
    bin_hT = [wsb.tile([P, R], MM_DT, tag=f"bin_hT{m}", name=f"{rp}bin_hT{m}")
              for m in range(2)]
    aT = [wsb.tile([P, R], F32, tag=f"aT{m}", name=f"{rp}aT{m}")
          for m in range(2)]
    for rc in range(R // RC):
        for m in range(2):
            p = psA.tile([P, RC], F32, tag="psA", name=f"{rp}pA")
            for k in range(4):
                nc.tensor.matmul(
                    p[:], bp_w[k][:, m * P:(m + 1) * P],
                    binT[k][:, rc * RC:(rc + 1) * RC],
                    start=(k == 0), stop=(k == 3))
            nc.vector.tensor_scalar_add(
                bin_hT[m][:, rc * RC:(rc + 1) * RC], p[:], bp_b[m][:])
        # aT[f, r] = sum_h w1a[h, f] * bin_hT[h, r]
        for m in range(2):
            p = psA.tile([P, RC], F32, tag="psA", name=f"{rp}pA")
            for k in range(2):
                nc.tensor.matmul(
                    p[:], w1a[k][:, m * P:(m + 1) * P],
                    bin_hT[k][:, rc * RC:(rc + 1) * RC],
                    start=(k == 0), stop=(k == 1))
            nc.vector.tensor_copy(aT[m][:, rc * RC:(rc + 1) * RC], p[:])

    # unit_hT[h, n] = sum_d up_w[d, h] * unitT[d, n]  (+ up_b)
    unit_hT = [wsb.tile([P, NPC], MM_DT, tag=f"unit_hT{m}", name=f"{rp}unit_hT{m}")
               for m in range(2)]
    for m in range(2):
        p = psA.tile([P, RC], F32, tag="psA", name=f"{rp}pA")
        for k in range(4):
            nc.tensor.matmul(
                p[:, :NPC], up_w[k][:, m * P:(m + 1) * P], unitT[k][:],
                start=(k == 0), stop=(k == 3))
        nc.vector.tensor_scalar_add(unit_hT[m][:], p[:, :NPC], up_b[m][:])

    # uT[f, n] = sum_h w1b[h, f] * unit_hT[h, n]  (+ b1)
    uT = [wsb.tile([P, NPC], F32, tag=f"uT{m}", name=f"{rp}uT{m}")
          for m in range(2)]
    for m in range(2):
        p = psA.tile([P, RC], F32, tag="psA", name=f"{rp}pA")
        for k in range(2):
            nc.tensor.matmul(
                p[:, :NPC], w1b[k][:, m * P:(m + 1) * P], unit_hT[k][:],
                start=(k == 0), stop=(k == 1))
        nc.vector.tensor_scalar_add(uT[m][:], p[:, :NPC], b1v[m][:])

    # ---- stage B: per-neuron MLP ----
    ps_out = psO.tile([P, 8 * NPC], F32, tag="ps_out", name=f"{rp}ps_out")

    GN = 4  # neurons per gelu1 group
    for grp in range(NPC // GN):
        # z[f, j*R + r] = aT[f, r] + uT[f, grp*GN+j]  (DVE), then one big gelu
        h1 = []
        for m in range(2):
            add_eng = nc.vector
            z = act.tile([P, GN * R], F32, tag=f"zh{m}", name=f"{rp}z{m}_{grp}")
            for j in range(GN):
                add_eng.tensor_scalar_add(
                    z[:, j * R:(j + 1) * R], aT[m][:],
                    uT[m][:, grp * GN + j: grp * GN + j + 1])
            h = act.tile([P, GN * R], MM_DT, tag=f"zh{m}", name=f"{rp}h1_{m}_{grp}")
            nc.scalar.activation(h[:], z[:], GELU)
            h1.append(h)

        for j in range(GN):
            n = grp * GN + j
            # h2pre[g, r] = sum_f w2[f, g] h1[f, r]
            p2 = ps2.tile([P, R], F32, tag="p2", name=f"{rp}p2_{n}")
            for rc in range(R // RC):
                for k in range(2):
                    nc.tensor.matmul(
                        p2[:, rc * RC:(rc + 1) * RC], w2[k][:],
                        h1[k][:, j * R + rc * RC: j * R + (rc + 1) * RC],
                        start=(k == 0), stop=(k == 1))
            # h2 = gelu(h2pre + b2)
            h2 = h2p.tile([P, R], H2_DT, tag="h2", name=f"{rp}h2_{n}")
            nc.scalar.activation(h2[:], p2[:], GELU, bias=b2v[:])
            # out[:, rt*32+n] = h2[:, rt*128:...]^T @ w3
            for rt in range(8):
                nc.tensor.matmul(
                    ps_out[:, rt * NPC + n: rt * NPC + n + 1],
                    h2[:, rt * P:(rt + 1) * P], w3[:],
                    start=True, stop=True)

    # ---- epilogue: +b3, clip, store ----
    ob = wsb.tile([P, 8 * NPC], F32, tag="ob", name=f"{rp}ob")
    nc.vector.tensor_scalar(ob[:], ps_out[:], b3v[:], -10.0,
                            op0=ALU.add, op1=ALU.max)
    nc.vector.tensor_scalar_min(ob[:], ob[:], 10.0)
    nc.sync.dma_start(
        out_d.rearrange("(t p) n -> p t n", p=P),
        ob[:].rearrange("p (t n) -> p t n", t=8))


def build_program(reps=1):
    nc = bacc.Bacc("TRN2", target_bir_lowering=False, debug=False,
                   num_devices=N_CORES)

    d = {}
    d["binT"] = nc.dram_tensor("binT", [DIM, R], MM_DT, kind="ExternalInput").ap()
    d["unitT"] = nc.dram_tensor("unitT", [DIM, NPC], MM_DT, kind="ExternalInput").ap()
    d["bp_w"] = nc.dram_tensor("bp_w", [DIM, HALF], MM_DT, kind="ExternalInput").ap()
    d["up_w"] = nc.dram_tensor("up_w", [DIM, HALF], MM_DT, kind="ExternalInput").ap()
    d["w1a"] = nc.dram_tensor("w1a", [HALF, HALF], MM_DT, kind="ExternalInput").ap()
    d["w1b"] = nc.dram_tensor("w1b", [HALF, HALF], MM_DT, kind="ExternalInput").ap()
    d["w2"] = nc.dram_tensor("w2", [HALF, QUART], MM_DT, kind="ExternalInput").ap()
    d["w3"] = nc.dram_tensor("w3", [QUART, 1], H2_DT, kind="ExternalInput").ap()
    d["bp_b"] = nc.dram_tensor("bp_b", [2, P, 1], F32, kind="ExternalInput").ap()
    d["up_b"] = nc.dram_tensor("up_b", [2, P, 1], F32, kind="ExternalInput").ap()
    d["b1v"] = nc.dram_tensor("b1v", [2, P, 1], F32, kind="ExternalInput").ap()
    d["b2v"] = nc.dram_tensor("b2v", [P, 1], F32, kind="ExternalInput").ap()
    d["b3v"] = nc.dram_tensor("b3v", [P, 1], F32, kind="ExternalInput").ap()
    out_d = nc.dram_tensor("out", [R, NPC], F32, kind="ExternalOutput").ap()

    with tile.TileContext(nc) as tc:
        with ExitStack() as ctx:
            wsb = ctx.enter_context(tc.tile_pool(name="wsb", bufs=1))
            act = ctx.enter_context(tc.tile_pool(name="act", bufs=3))
            h2p = ctx.enter_context(tc.tile_pool(name="h2p", bufs=2))
            psA = ctx.enter_context(tc.tile_pool(name="psA", bufs=2, space="PSUM"))
            ps2 = ctx.enter_context(tc.tile_pool(name="ps2", bufs=2, space="PSUM"))
            psO = ctx.enter_context(tc.tile_pool(name="psO", bufs=1, space="PSUM"))
            pools = (wsb, act, h2p, psA, ps2, psO)
            for rep in range(reps):
                _build_body(nc, tc, pools, d, out_d, rep)

    nc.compile()
    return nc


def _make_in_maps(bin_repr, unit_embs, bp_w, bp_b, up_w, up_b, w1, b1, w2, b2,
                  w3, b3):
    f32 = np.float32
    w3_np = np.ascontiguousarray(w3, f32)
    if H2_DT == BF16:
        w3_np = w3_np.astype(ml_dtypes.bfloat16)
    binT = np.ascontiguousarray(bin_repr.reshape(R, DIM).T).astype(f32)
    common = {
        "binT": binT,
        "bp_w": np.ascontiguousarray(bp_w, f32),
        "up_w": np.ascontiguousarray(up_w, f32),
        "w1a": np.ascontiguousarray(w1[:HALF], f32),
        "w1b": np.ascontiguousarray(w1[HALF:], f32),
        "w2": np.ascontiguousarray(w2, f32),
        "w3": w3_np,
        "bp_b": np.ascontiguousarray(bp_b, f32).reshape(2, P, 1),
        "up_b": np.ascontiguousarray(up_b, f32).reshape(2, P, 1),
        "b1v": np.ascontiguousarray(b1, f32).reshape(2, P, 1),
        "b2v": np.ascontiguousarray(b2, f32).reshape(P, 1),
        "b3v": np.full((P, 1), np.float32(np.asarray(b3).reshape(-1)[0]), f32),
    }
    in_maps = []
    for c in range(N_CORES):
        m = dict(common)
        m["unitT"] = np.ascontiguousarray(
            unit_embs[c * NPC:(c + 1) * NPC].T).astype(f32)
        in_maps.append(m)
    return in_maps


def _gather(res):
    parts = [res.results[c]["out"] for c in range(N_CORES)]  # each [R, NPC]
    full = np.concatenate(parts, axis=1)                     # [R, N]
    return full.reshape(B, T, N_NEURONS).astype(np.float32)


def kernel(**inputs):
    if "nc" not in _CACHE:
        _CACHE["nc"] = build_program()
    in_maps = _make_in_maps(**{k: np.asarray(v) for k, v in inputs.items()})
    res = run_bass_kernel_spmd(_CACHE["nc"], in_maps,
                               core_ids=list(range(N_CORES)))
    return _gather(res)
